# revision 20
# baseline (speedup 1.0000x reference)
"""Multi-head attention (B=8, S=2048, D=256, H=4) on 8 TRN2 NeuronCores.

Sharding: pure data-parallel over batch — core c computes batch element c.
Weights are replicated; no collectives.

Per-core kernel (x: [S, D]):
  xT   = transpose(x)                      (PE transposes, 128x128 tiles)
  qT   = w_q^T @ xT    [D, S]  (transposed layout)
  kT   = w_k^T @ xT    [D, S]
  v    = x @ w_v       [S, D]  (natural layout, stored as vp = [v_h | 1] per head)
  per head h, per sq half:
    scoresT[sk, sq] = kT_h^T(sk-tile) . qT_h   (PE; K=64)
    expT = exp(0.125 * scoresT)                (ACT, one pass, no max-sub:
                                                softmax is shift-invariant and
                                                |s/8| is O(10) => fp32-safe)
    ctxT[65, sq] += vp_h^T(sk-tile) @ expT     (PE; row 64 = softmax denom via
                                                the ones column of vp)
    ctxT_sb = ctxT[0:64] * bcast(1/ctxT[64])   (PE rank-1 bcast + DVE mul;
                                                deferred into the next unit's
                                                sk-loop so PE never stalls on
                                                the 1-lane reciprocal)
  out  = ctx @ w_o + b_o                       (PE over ctxT chunks + DVE add)

Matmul operands are MM_DTYPE (bf16 by default: fp32 matmul on TRN2 runs
2-pass LOW_HIGH with serial un-hidden LDWEIGHTS — ~3x slower). PSUM
accumulation, softmax denominators, reciprocal, broadcast, and the bias
add are always fp32.
"""

import numpy as np

import concourse.bass as bass
import concourse.mybir as mybir
import concourse.tile as tile
from concourse import bacc, bass_utils

B, S, D = 8, 2048, 256
H = 4
DH = D // H          # 64
P = 128              # partitions
SCALE = 1.0 / np.sqrt(DH)  # 0.125
N_CORES = 8

FP32 = mybir.dt.float32
BF16 = mybir.dt.bfloat16
# scores path dtype (x/w_q/w_k/qT/kT) and PV path dtype (vp/expT/ctxT/w_o)
SCORES_DTYPE = BF16
PV_DTYPE = BF16


def build_bass(scores_dtype=None, pv_dtype=None):
    if scores_dtype is None:
        scores_dtype = SCORES_DTYPE
    if pv_dtype is None:
        pv_dtype = PV_DTYPE
    nc = bacc.Bacc("TRN2", target_bir_lowering=False, debug=False,
                   num_devices=N_CORES)

    x_d = nc.dram_tensor("x", [S, D], FP32, kind="ExternalInput").ap()
    wq_d = nc.dram_tensor("w_q", [D, D], FP32, kind="ExternalInput").ap()
    wk_d = nc.dram_tensor("w_k", [D, D], FP32, kind="ExternalInput").ap()
    wv_d = nc.dram_tensor("w_v", [D, D], FP32, kind="ExternalInput").ap()
    wo_d = nc.dram_tensor("w_o", [D, D], FP32, kind="ExternalInput").ap()
    bo_d = nc.dram_tensor("b_o", [1, D], FP32, kind="ExternalInput").ap()
    out_d = nc.dram_tensor("out", [S, D], FP32, kind="ExternalOutput").ap()

    n_sq = S // P        # 16 seq tiles
    n_kc = D // P        # 2 contraction chunks of 128
    NQ = 1024            # sq chunk width for attention inner loop
    n_sqh = S // NQ      # 2

    with tile.TileContext(nc) as tc:
        with (
            tc.tile_pool(name="const", bufs=1) as const_pool,
            tc.tile_pool(name="bigbuf", bufs=1) as big_pool,
            tc.tile_pool(name="xin", bufs=16) as xin_pool,
            tc.tile_pool(name="drain", bufs=3) as drain_pool,
            tc.tile_pool(name="expbuf", bufs=3) as exp_pool,
            tc.tile_pool(name="outbuf", bufs=3) as out_pool,
        ):
            # ---- constants ----
            ident = const_pool.tile([P, P], FP32)
            from concourse.masks import make_identity
            make_identity(nc, ident)
            ones_sb = const_pool.tile([1, DH], FP32)
            nc.vector.memset(ones_sb, 1.0)

            # x tile loads go FIRST (they gate the transpose/projection
            # chain), split across two HWDGE queues (sync + scalar) so the
            # ~0.6us per-DMA issue cost pipelines 2-wide.
            x_tiles = []
            for t in range(n_sq):
                xt = xin_pool.tile([P, D], FP32, tag="xtile",
                                   name=f"xt{t}")
                eng = nc.sync if t % 2 == 0 else nc.scalar
                eng.dma_start(out=xt, in_=x_d[t * P:(t + 1) * P, :])
                x_tiles.append(xt)

            w_sb = {}
            w_dt = {"wq": scores_dtype, "wk": scores_dtype,
                    "wv": scores_dtype, "wo": pv_dtype}
            for name, dram in (("wq", wq_d), ("wk", wk_d), ("wv", wv_d),
                               ("wo", wo_d)):
                for kc in range(n_kc):
                    tf = const_pool.tile([P, D], FP32, name=f"{name}f{kc}")
                    nc.scalar.dma_start(
                        out=tf, in_=dram[kc * P:(kc + 1) * P, :])
                    if w_dt[name] == FP32:
                        w_sb[name, kc] = tf
                    else:
                        t = const_pool.tile([P, D], w_dt[name],
                                            name=f"{name}{kc}")
                        nc.vector.tensor_copy(t, tf)
                        w_sb[name, kc] = t

            # bias broadcast to all 128 partitions (DMA stride-0 read)
            bias_sb = const_pool.tile([P, D], FP32)
            nc.scalar.dma_start(out=bias_sb, in_=bo_d.partition_broadcast(P))

            # ---- persistent big SBUF tensors (mm_dtype) ----
            xT = [big_pool.tile([P, S], scores_dtype, name=f"xT{c}")
                  for c in range(n_kc)]
            qT = [big_pool.tile([P, S], scores_dtype, name=f"qT{c}")
                  for c in range(n_kc)]
            kT = [big_pool.tile([P, S], scores_dtype, name=f"kT{c}")
                  for c in range(n_kc)]
            # vp[t]: [128, 4*128]; per head h cols h*128..h*128+63 = v_h,
            # col h*128+64 = ones (softmax denominator), rest = padding so the
            # PV matmul is a full-128-column stationary like the scores
            # matmul (uniform shape -> back-to-back PE streaming, no
            # row-group reconfig flush between score and PV pairs)
            vp = [big_pool.tile([P, H * P], pv_dtype, name=f"vp{t}")
                  for t in range(n_sq)]
            ctxT = [big_pool.tile([P, S], pv_dtype, name=f"ctxT{c}")
                    for c in range(n_kc)]

            # ---- stage 1: transpose x into xT (cast at drain) ----
            with tc.tile_pool(name="pt", bufs=4, space="PSUM") as pt_pool:
                for t in range(n_sq):
                    xt = x_tiles[t]
                    for c in range(n_kc):
                        ps = pt_pool.tile([P, P], FP32, tag="tr")
                        nc.tensor.transpose(ps, xt[:, c * P:(c + 1) * P], ident)
                        nc.vector.tensor_copy(
                            xT[c][:, t * P:(t + 1) * P], ps)

            # ---- stage 2: projections ----
            with tc.tile_pool(name="pp", bufs=2, space="PSUM") as pp_pool:
                # qT / kT: out[do_chunk][:, :] = sum_kc w[kc][:, do].T @ xT[kc]
                for name, dest in (("wq", qT), ("wk", kT)):
                    for do in range(n_kc):
                        for half in range(2):
                            ps = pp_pool.tile([P, NQ], FP32, tag="proj")
                            for kc in range(n_kc):
                                for nck in range(2):
                                    off = half * NQ + nck * 512
                                    nc.tensor.matmul(
                                        ps[:, nck * 512:(nck + 1) * 512],
                                        lhsT=w_sb[name, kc][:, do * P:(do + 1) * P],
                                        rhs=xT[kc][:, off:off + 512],
                                        start=(kc == 0), stop=(kc == n_kc - 1))
                            nc.vector.tensor_copy(
                                dest[do][:, half * NQ:(half + 1) * NQ], ps)
                # v (natural layout) -> vp tiles
                for t in range(n_sq):
                    ps = pp_pool.tile([P, D], FP32, tag="vproj")
                    for kc in range(n_kc):
                        nc.tensor.matmul(
                            ps, lhsT=xT[kc][:, t * P:(t + 1) * P],
                            rhs=w_sb["wv", kc],
                            start=(kc == 0), stop=(kc == n_kc - 1))
                    nc.vector.memset(vp[t], 1.0)
                    for h in range(H):
                        nc.vector.tensor_copy(
                            vp[t][:, h * P:h * P + DH],
                            ps[:, h * DH:(h + 1) * DH])

            # ---- stage 3: attention ----
            with (
                tc.tile_pool(name="psc", bufs=2, space="PSUM") as sc_pool,
                tc.tile_pool(name="pctx", bufs=2, space="PSUM") as ctx_pool,
                tc.tile_pool(name="dscr", bufs=2, space="DRAM") as dram_pool,
            ):
                # Software pipeline with lag-1 between scores and PV: the PE
                # FIFO executes in order, so PV(k) (which waits on exp(k) from
                # ACT) must sit BEHIND scores(k+1) in the queue — otherwise
                # every step serializes PE -> ACT -> PE and both engines idle.
                def emit_drain(h, q0, ctx_ps):
                    # normalize rows 0..63 by 1/row64. Reciprocal is a slow
                    # 1-lane DVE op and the broadcast goes via a DRAM
                    # round-trip (stride-0 DRAM reads are legal) — all off
                    # the PE critical path; ctx_pool bufs=2 gives a full
                    # unit of slack before the slot is needed again.
                    recip = drain_pool.tile([1, NQ], FP32, tag="recip")
                    nc.vector.reciprocal(recip, ctx_ps[DH:DH + 1, :])
                    rscr = dram_pool.tile([1, NQ], FP32, tag="rscr")
                    nc.sync.dma_start(out=rscr, in_=recip)
                    recip_b = drain_pool.tile([DH, NQ], FP32, tag="recipb")
                    nc.sync.dma_start(
                        out=recip_b, in_=rscr.partition_broadcast(DH))
                    nc.vector.tensor_mul(
                        ctxT[h // 2][(h % 2) * DH:(h % 2) * DH + DH,
                                     q0:q0 + NQ],
                        ctx_ps[0:DH, :], recip_b)

                steps = [(h, sqh, sk)
                         for h in range(H)
                         for sqh in range(n_sqh)
                         for sk in range(n_sq)]
                ctx_cur = [None]
                pending_pv = [None]

                def emit_pv():
                    if pending_pv[0] is None:
                        return
                    ex, h, sk, q0, ctx_ps = pending_pv[0]
                    pending_pv[0] = None
                    for nck in range(2):
                        nc.tensor.matmul(
                            ctx_ps[:, nck * 512:(nck + 1) * 512],
                            lhsT=vp[sk][:, h * P:(h + 1) * P],
                            rhs=ex[:, nck * 512:(nck + 1) * 512],
                            start=(sk == 0), stop=(sk == n_sq - 1))
                    if sk == n_sq - 1:
                        emit_drain(h, q0, ctx_ps)

                for (h, sqh, sk) in steps:
                    qTh = qT[h // 2][(h % 2) * DH:(h % 2) * DH + DH, :]
                    kTh = kT[h // 2][(h % 2) * DH:(h % 2) * DH + DH, :]
                    q0 = sqh * NQ
                    if sk == 0:
                        ctx_cur[0] = ctx_pool.tile(
                            [P, NQ], FP32, tag="ctx",
                            name=f"ctxps_{h}_{sqh}")
                    ctx_ps = ctx_cur[0]
                    sc = sc_pool.tile([P, NQ], FP32, tag="sc")
                    for nck in range(2):
                        nc.tensor.matmul(
                            sc[:, nck * 512:(nck + 1) * 512],
                            lhsT=kTh[:, sk * P:(sk + 1) * P],
                            rhs=qTh[:, q0 + nck * 512:q0 + (nck + 1) * 512],
                            start=True, stop=True)
                    ex = exp_pool.tile([P, NQ], pv_dtype, tag="exp")
                    nc.scalar.activation(
                        ex, sc, mybir.ActivationFunctionType.Exp,
                        scale=float(SCALE))
                    emit_pv()
                    pending_pv[0] = (ex, h, sk, q0, ctx_ps)
                emit_pv()

            # ---- stage 4: output projection + bias ----
            with tc.tile_pool(name="po", bufs=3, space="PSUM") as po_pool:
                for t in range(n_sq):
                    ps = po_pool.tile([P, D], FP32, tag="oproj")
                    for kc in range(n_kc):
                        nc.tensor.matmul(
                            ps, lhsT=ctxT[kc][:, t * P:(t + 1) * P],
                            rhs=w_sb["wo", kc],
                            start=(kc == 0), stop=(kc == n_kc - 1))
                    ot = out_pool.tile([P, D], FP32, tag="otile")
                    nc.vector.tensor_add(ot, ps, bias_sb)
                    nc.sync.dma_start(
                        out=out_d[t * P:(t + 1) * P, :], in_=ot)

    nc.compile()
    return nc


_NC_CACHE = None


def kernel(x, w_q, w_k, w_v, w_o, b_o):
    global _NC_CACHE
    x = np.ascontiguousarray(np.asarray(x), dtype=np.float32)
    w_q = np.ascontiguousarray(np.asarray(w_q), dtype=np.float32)
    w_k = np.ascontiguousarray(np.asarray(w_k), dtype=np.float32)
    w_v = np.ascontiguousarray(np.asarray(w_v), dtype=np.float32)
    w_o = np.ascontiguousarray(np.asarray(w_o), dtype=np.float32)
    b_o = np.ascontiguousarray(np.asarray(b_o), dtype=np.float32).reshape(1, D)

    if _NC_CACHE is None:
        _NC_CACHE = build_bass()
    nc = _NC_CACHE

    in_maps = []
    for c in range(N_CORES):
        in_maps.append({
            "x": np.ascontiguousarray(x[c]),
            "w_q": w_q, "w_k": w_k, "w_v": w_v, "w_o": w_o, "b_o": b_o,
        })

    res = bass_utils.run_bass_kernel_spmd(
        nc, in_maps, core_ids=list(range(N_CORES)))
    global _LAST_RESULTS
    _LAST_RESULTS = res
    out = np.stack([res.results[c]["out"] for c in range(N_CORES)], axis=0)
    return out


_LAST_RESULTS = None


if __name__ == "__main__":
    rng = np.random.default_rng(0)
    ins = {
        "x": rng.standard_normal((B, S, D), dtype=np.float32),
        "w_q": rng.standard_normal((D, D), dtype=np.float32) / 16,
        "w_k": rng.standard_normal((D, D), dtype=np.float32) / 16,
        "w_v": rng.standard_normal((D, D), dtype=np.float32) / 16,
        "w_o": rng.standard_normal((D, D), dtype=np.float32) / 16,
        "b_o": rng.standard_normal((D,), dtype=np.float32) * 0.01,
    }
    out = kernel(**ins)
    print(out.shape, out.dtype)


# revision 22
# speedup vs baseline: 1.1666x; 1.1666x over previous
"""Multi-head attention (B=8, S=2048, D=256, H=4) on 8 TRN2 NeuronCores.

Sharding: pure data-parallel over batch — core c computes batch element c.
Weights are replicated; no collectives.

Per-core kernel (x: [S, D]):
  xT   = transpose(x)                      (PE transposes, 128x128 tiles)
  qT   = w_q^T @ xT    [D, S]  (transposed layout)
  kT   = w_k^T @ xT    [D, S]
  v    = x @ w_v       [S, D]  (natural layout, stored as vp = [v_h | 1] per head)
  per head h, per sq half:
    scoresT[sk, sq] = kT_h^T(sk-tile) . qT_h   (PE; K=64)
    expT = exp(0.125 * scoresT)                (ACT, one pass, no max-sub:
                                                softmax is shift-invariant and
                                                |s/8| is O(10) => fp32-safe)
    ctxT[65, sq] += vp_h^T(sk-tile) @ expT     (PE; row 64 = softmax denom via
                                                the ones column of vp)
    ctxT_sb = ctxT[0:64] * bcast(1/ctxT[64])   (PE rank-1 bcast + DVE mul;
                                                deferred into the next unit's
                                                sk-loop so PE never stalls on
                                                the 1-lane reciprocal)
  out  = ctx @ w_o + b_o                       (PE over ctxT chunks + DVE add)

Matmul operands are MM_DTYPE (bf16 by default: fp32 matmul on TRN2 runs
2-pass LOW_HIGH with serial un-hidden LDWEIGHTS — ~3x slower). PSUM
accumulation, softmax denominators, reciprocal, broadcast, and the bias
add are always fp32.
"""

import numpy as np

import concourse.bass as bass
import concourse.mybir as mybir
import concourse.tile as tile
from concourse import bacc, bass_utils

B, S, D = 8, 2048, 256
H = 4
DH = D // H          # 64
P = 128              # partitions
SCALE = 1.0 / np.sqrt(DH)  # 0.125
N_CORES = 8

FP32 = mybir.dt.float32
BF16 = mybir.dt.bfloat16
# scores path dtype (x/w_q/w_k/qT/kT) and PV path dtype (vp/expT/ctxT/w_o)
SCORES_DTYPE = BF16
PV_DTYPE = BF16


def build_bass(scores_dtype=None, pv_dtype=None):
    if scores_dtype is None:
        scores_dtype = SCORES_DTYPE
    if pv_dtype is None:
        pv_dtype = PV_DTYPE
    nc = bacc.Bacc("TRN2", target_bir_lowering=False, debug=False,
                   num_devices=N_CORES)

    x_d = nc.dram_tensor("x", [S, D], FP32, kind="ExternalInput").ap()
    wq_d = nc.dram_tensor("w_q", [D, D], FP32, kind="ExternalInput").ap()
    wk_d = nc.dram_tensor("w_k", [D, D], FP32, kind="ExternalInput").ap()
    wv_d = nc.dram_tensor("w_v", [D, D], FP32, kind="ExternalInput").ap()
    wo_d = nc.dram_tensor("w_o", [D, D], FP32, kind="ExternalInput").ap()
    bo_d = nc.dram_tensor("b_o", [1, D], FP32, kind="ExternalInput").ap()
    out_d = nc.dram_tensor("out", [S, D], FP32, kind="ExternalOutput").ap()

    n_sq = S // P        # 16 seq tiles
    n_kc = D // P        # 2 contraction chunks of 128
    NQ = 1024            # sq chunk width for attention inner loop
    n_sqh = S // NQ      # 2

    with tile.TileContext(nc) as tc:
        with (
            tc.tile_pool(name="const", bufs=1) as const_pool,
            tc.tile_pool(name="bigbuf", bufs=1) as big_pool,
            tc.tile_pool(name="xin", bufs=16) as xin_pool,
            tc.tile_pool(name="drain", bufs=3) as drain_pool,
            tc.tile_pool(name="expbuf", bufs=3) as exp_pool,
            tc.tile_pool(name="outbuf", bufs=3) as out_pool,
        ):
            # ---- constants ----
            ident = const_pool.tile([P, P], FP32)
            from concourse.masks import make_identity
            make_identity(nc, ident)
            ones_sb = const_pool.tile([1, DH], FP32)
            nc.vector.memset(ones_sb, 1.0)

            # x tile loads go FIRST (they gate the transpose/projection
            # chain), split across two HWDGE queues (sync + scalar) so the
            # ~0.6us per-DMA issue cost pipelines 2-wide. Weight loads are
            # interleaved on the scalar queue after the first half of x.
            x_tiles = [None] * n_sq
            for t in range(0, n_sq, 2):
                xt = xin_pool.tile([P, D], FP32, tag="xtile", name=f"xt{t}")
                nc.sync.dma_start(out=xt, in_=x_d[t * P:(t + 1) * P, :])
                x_tiles[t] = xt
            for t in range(1, 8, 2):
                xt = xin_pool.tile([P, D], FP32, tag="xtile", name=f"xt{t}")
                nc.scalar.dma_start(out=xt, in_=x_d[t * P:(t + 1) * P, :])
                x_tiles[t] = xt

            w_sb = {}
            w_dt = {"wq": scores_dtype, "wk": scores_dtype,
                    "wv": scores_dtype, "wo": pv_dtype}
            w_f = {}
            for name, dram in (("wq", wq_d), ("wk", wk_d), ("wv", wv_d),
                               ("wo", wo_d)):
                for kc in range(n_kc):
                    tf = const_pool.tile([P, D], FP32, name=f"{name}f{kc}")
                    nc.scalar.dma_start(
                        out=tf, in_=dram[kc * P:(kc + 1) * P, :])
                    w_f[name, kc] = tf
            for t in range(9, n_sq, 2):
                xt = xin_pool.tile([P, D], FP32, tag="xtile", name=f"xt{t}")
                nc.scalar.dma_start(out=xt, in_=x_d[t * P:(t + 1) * P, :])
                x_tiles[t] = xt
            for (name, kc), tf in w_f.items():
                if w_dt[name] == FP32:
                    w_sb[name, kc] = tf
                else:
                    t = const_pool.tile([P, D], w_dt[name], name=f"{name}{kc}")
                    nc.gpsimd.tensor_copy(t, tf)
                    w_sb[name, kc] = t

            # bias broadcast to all 128 partitions (DMA stride-0 read)
            bias_sb = const_pool.tile([P, D], FP32)
            nc.scalar.dma_start(out=bias_sb, in_=bo_d.partition_broadcast(P))

            # ---- persistent big SBUF tensors (mm_dtype) ----
            xT = [big_pool.tile([P, S], scores_dtype, name=f"xT{c}")
                  for c in range(n_kc)]
            qT = [big_pool.tile([P, S], scores_dtype, name=f"qT{c}")
                  for c in range(n_kc)]
            kT = [big_pool.tile([P, S], scores_dtype, name=f"kT{c}")
                  for c in range(n_kc)]
            # vp[t]: [128, 4*65]; per head h cols h*65..h*65+64 = [v_h | 1]
            vp = [big_pool.tile([P, H * (DH + 1)], pv_dtype, name=f"vp{t}")
                  for t in range(n_sq)]
            ctxT = [big_pool.tile([P, S], pv_dtype, name=f"ctxT{c}")
                    for c in range(n_kc)]

            # ---- stage 1: transpose x into xT (cast at drain) ----
            with tc.tile_pool(name="pt", bufs=4, space="PSUM") as pt_pool:
                for t in range(n_sq):
                    xt = x_tiles[t]
                    for c in range(n_kc):
                        ps = pt_pool.tile([P, P], FP32, tag="tr")
                        nc.tensor.transpose(ps, xt[:, c * P:(c + 1) * P], ident)
                        nc.scalar.copy(
                            xT[c][:, t * P:(t + 1) * P], ps)

            # ---- stage 2: projections ----
            with tc.tile_pool(name="pp", bufs=2, space="PSUM") as pp_pool:
                # qT / kT: out[do_chunk][:, :] = sum_kc w[kc][:, do].T @ xT[kc]
                for name, dest in (("wq", qT), ("wk", kT)):
                    for do in range(n_kc):
                        for half in range(2):
                            ps = pp_pool.tile([P, NQ], FP32, tag="proj")
                            for kc in range(n_kc):
                                for nck in range(2):
                                    off = half * NQ + nck * 512
                                    nc.tensor.matmul(
                                        ps[:, nck * 512:(nck + 1) * 512],
                                        lhsT=w_sb[name, kc][:, do * P:(do + 1) * P],
                                        rhs=xT[kc][:, off:off + 512],
                                        start=(kc == 0), stop=(kc == n_kc - 1))
                            nc.vector.tensor_copy(
                                dest[do][:, half * NQ:(half + 1) * NQ], ps)
                # v (natural layout) -> vp tiles
                for t in range(n_sq):
                    ps = pp_pool.tile([P, D], FP32, tag="vproj")
                    for kc in range(n_kc):
                        nc.tensor.matmul(
                            ps, lhsT=xT[kc][:, t * P:(t + 1) * P],
                            rhs=w_sb["wv", kc],
                            start=(kc == 0), stop=(kc == n_kc - 1))
                    nc.gpsimd.memset(vp[t], 1.0)
                    for h in range(H):
                        nc.vector.tensor_copy(
                            vp[t][:, h * (DH + 1):h * (DH + 1) + DH],
                            ps[:, h * DH:(h + 1) * DH])

            # ---- stage 3: attention ----
            with (
                tc.tile_pool(name="psc", bufs=2, space="PSUM") as sc_pool,
                tc.tile_pool(name="pctx", bufs=2, space="PSUM") as ctx_pool,
                tc.tile_pool(name="dscr", bufs=2, space="DRAM") as dram_pool,
            ):
                # Software pipeline with lag-1 between scores and PV: the PE
                # FIFO executes in order, so PV(k) (which waits on exp(k) from
                # ACT) must sit BEHIND scores(k+1) in the queue — otherwise
                # every step serializes PE -> ACT -> PE and both engines idle.
                def emit_drain(h, q0, ctx_ps):
                    # normalize rows 0..63 by 1/row64. Reciprocal is a slow
                    # 1-lane DVE op and the broadcast goes via a DRAM
                    # round-trip (stride-0 DRAM reads are legal) — all off
                    # the PE critical path; ctx_pool bufs=2 gives a full
                    # unit of slack before the slot is needed again.
                    recip = drain_pool.tile([1, NQ], FP32, tag="recip")
                    nc.vector.reciprocal(recip, ctx_ps[DH:DH + 1, :])
                    rscr = dram_pool.tile([1, NQ], FP32, tag="rscr")
                    nc.sync.dma_start(out=rscr, in_=recip)
                    recip_b = drain_pool.tile([DH, NQ], FP32, tag="recipb")
                    nc.sync.dma_start(
                        out=recip_b, in_=rscr.partition_broadcast(DH))
                    nc.vector.tensor_mul(
                        ctxT[h // 2][(h % 2) * DH:(h % 2) * DH + DH,
                                     q0:q0 + NQ],
                        ctx_ps[0:DH, :], recip_b)

                steps = [(h, sqh, sk)
                         for h in range(H)
                         for sqh in range(n_sqh)
                         for sk in range(n_sq)]
                ctx_cur = [None]
                pending_pv = [None]

                def emit_pv():
                    if pending_pv[0] is None:
                        return
                    ex, h, sk, q0, ctx_ps = pending_pv[0]
                    pending_pv[0] = None
                    for nck in range(2):
                        nc.tensor.matmul(
                            ctx_ps[:, nck * 512:(nck + 1) * 512],
                            lhsT=vp[sk][:, h * (DH + 1):(h + 1) * (DH + 1)],
                            rhs=ex[:, nck * 512:(nck + 1) * 512],
                            start=(sk == 0), stop=(sk == n_sq - 1))
                    if sk == n_sq - 1:
                        emit_drain(h, q0, ctx_ps)

                for (h, sqh, sk) in steps:
                    qTh = qT[h // 2][(h % 2) * DH:(h % 2) * DH + DH, :]
                    kTh = kT[h // 2][(h % 2) * DH:(h % 2) * DH + DH, :]
                    q0 = sqh * NQ
                    if sk == 0:
                        ctx_cur[0] = ctx_pool.tile(
                            [DH + 1, NQ], FP32, tag="ctx",
                            name=f"ctxps_{h}_{sqh}")
                    ctx_ps = ctx_cur[0]
                    sc = sc_pool.tile([P, NQ], FP32, tag="sc")
                    for nck in range(2):
                        nc.tensor.matmul(
                            sc[:, nck * 512:(nck + 1) * 512],
                            lhsT=kTh[:, sk * P:(sk + 1) * P],
                            rhs=qTh[:, q0 + nck * 512:q0 + (nck + 1) * 512],
                            start=True, stop=True)
                    ex = exp_pool.tile([P, NQ], pv_dtype, tag="exp")
                    nc.scalar.activation(
                        ex, sc, mybir.ActivationFunctionType.Exp,
                        scale=float(SCALE))
                    emit_pv()
                    pending_pv[0] = (ex, h, sk, q0, ctx_ps)
                emit_pv()

            # ---- stage 4: output projection + bias ----
            with tc.tile_pool(name="po", bufs=3, space="PSUM") as po_pool:
                for t in range(n_sq):
                    ps = po_pool.tile([P, D], FP32, tag="oproj")
                    for kc in range(n_kc):
                        nc.tensor.matmul(
                            ps, lhsT=ctxT[kc][:, t * P:(t + 1) * P],
                            rhs=w_sb["wo", kc],
                            start=(kc == 0), stop=(kc == n_kc - 1))
                    ot = out_pool.tile([P, D], FP32, tag="otile")
                    nc.vector.tensor_add(ot, ps, bias_sb)
                    nc.sync.dma_start(
                        out=out_d[t * P:(t + 1) * P, :], in_=ot)

    nc.compile()
    return nc


_NC_CACHE = None


def kernel(x, w_q, w_k, w_v, w_o, b_o):
    global _NC_CACHE
    x = np.ascontiguousarray(np.asarray(x), dtype=np.float32)
    w_q = np.ascontiguousarray(np.asarray(w_q), dtype=np.float32)
    w_k = np.ascontiguousarray(np.asarray(w_k), dtype=np.float32)
    w_v = np.ascontiguousarray(np.asarray(w_v), dtype=np.float32)
    w_o = np.ascontiguousarray(np.asarray(w_o), dtype=np.float32)
    b_o = np.ascontiguousarray(np.asarray(b_o), dtype=np.float32).reshape(1, D)

    if _NC_CACHE is None:
        _NC_CACHE = build_bass()
    nc = _NC_CACHE

    in_maps = []
    for c in range(N_CORES):
        in_maps.append({
            "x": np.ascontiguousarray(x[c]),
            "w_q": w_q, "w_k": w_k, "w_v": w_v, "w_o": w_o, "b_o": b_o,
        })

    res = bass_utils.run_bass_kernel_spmd(
        nc, in_maps, core_ids=list(range(N_CORES)))
    global _LAST_RESULTS
    _LAST_RESULTS = res
    out = np.stack([res.results[c]["out"] for c in range(N_CORES)], axis=0)
    return out


_LAST_RESULTS = None


if __name__ == "__main__":
    rng = np.random.default_rng(0)
    ins = {
        "x": rng.standard_normal((B, S, D), dtype=np.float32),
        "w_q": rng.standard_normal((D, D), dtype=np.float32) / 16,
        "w_k": rng.standard_normal((D, D), dtype=np.float32) / 16,
        "w_v": rng.standard_normal((D, D), dtype=np.float32) / 16,
        "w_o": rng.standard_normal((D, D), dtype=np.float32) / 16,
        "b_o": rng.standard_normal((D,), dtype=np.float32) * 0.01,
    }
    out = kernel(**ins)
    print(out.shape, out.dtype)


# revision 23
# speedup vs baseline: 1.1681x; 1.0013x over previous
"""Multi-head attention (B=8, S=2048, D=256, H=4) on 8 TRN2 NeuronCores.

Sharding: pure data-parallel over batch — core c computes batch element c.
Weights are replicated; no collectives.

Per-core kernel (x: [S, D]):
  xT   = transpose(x)                      (PE transposes, 128x128 tiles)
  qT   = w_q^T @ xT    [D, S]  (transposed layout)
  kT   = w_k^T @ xT    [D, S]
  v    = x @ w_v       [S, D]  (natural layout, stored as vp = [v_h | 1] per head)
  per head h, per sq half:
    scoresT[sk, sq] = kT_h^T(sk-tile) . qT_h   (PE; K=64)
    expT = exp(0.125 * scoresT)                (ACT, one pass, no max-sub:
                                                softmax is shift-invariant and
                                                |s/8| is O(10) => fp32-safe)
    ctxT[65, sq] += vp_h^T(sk-tile) @ expT     (PE; row 64 = softmax denom via
                                                the ones column of vp)
    ctxT_sb = ctxT[0:64] * bcast(1/ctxT[64])   (PE rank-1 bcast + DVE mul;
                                                deferred into the next unit's
                                                sk-loop so PE never stalls on
                                                the 1-lane reciprocal)
  out  = ctx @ w_o + b_o                       (PE over ctxT chunks + DVE add)

Matmul operands are MM_DTYPE (bf16 by default: fp32 matmul on TRN2 runs
2-pass LOW_HIGH with serial un-hidden LDWEIGHTS — ~3x slower). PSUM
accumulation, softmax denominators, reciprocal, broadcast, and the bias
add are always fp32.
"""

import numpy as np

import concourse.bass as bass
import concourse.mybir as mybir
import concourse.tile as tile
from concourse import bacc, bass_utils

B, S, D = 8, 2048, 256
H = 4
DH = D // H          # 64
P = 128              # partitions
SCALE = 1.0 / np.sqrt(DH)  # 0.125
N_CORES = 8

FP32 = mybir.dt.float32
BF16 = mybir.dt.bfloat16
# scores path dtype (x/w_q/w_k/qT/kT) and PV path dtype (vp/expT/ctxT/w_o)
SCORES_DTYPE = BF16
PV_DTYPE = BF16


def build_bass(scores_dtype=None, pv_dtype=None):
    if scores_dtype is None:
        scores_dtype = SCORES_DTYPE
    if pv_dtype is None:
        pv_dtype = PV_DTYPE
    nc = bacc.Bacc("TRN2", target_bir_lowering=False, debug=False,
                   num_devices=N_CORES)

    x_d = nc.dram_tensor("x", [S, D], FP32, kind="ExternalInput").ap()
    wq_d = nc.dram_tensor("w_q", [D, D], FP32, kind="ExternalInput").ap()
    wk_d = nc.dram_tensor("w_k", [D, D], FP32, kind="ExternalInput").ap()
    wv_d = nc.dram_tensor("w_v", [D, D], FP32, kind="ExternalInput").ap()
    wo_d = nc.dram_tensor("w_o", [D, D], FP32, kind="ExternalInput").ap()
    bo_d = nc.dram_tensor("b_o", [1, D], FP32, kind="ExternalInput").ap()
    out_d = nc.dram_tensor("out", [S, D], FP32, kind="ExternalOutput").ap()

    n_sq = S // P        # 16 seq tiles
    n_kc = D // P        # 2 contraction chunks of 128
    NQ = 1024            # sq chunk width for attention inner loop
    n_sqh = S // NQ      # 2

    with tile.TileContext(nc) as tc:
        with (
            tc.tile_pool(name="const", bufs=1) as const_pool,
            tc.tile_pool(name="bigbuf", bufs=1) as big_pool,
            tc.tile_pool(name="xin", bufs=16) as xin_pool,
            tc.tile_pool(name="drain", bufs=3) as drain_pool,
            tc.tile_pool(name="expbuf", bufs=3) as exp_pool,
            tc.tile_pool(name="outbuf", bufs=3) as out_pool,
        ):
            # ---- constants ----
            ident = const_pool.tile([P, P], FP32)
            from concourse.masks import make_identity
            make_identity(nc, ident)
            ones_sb = const_pool.tile([1, DH], FP32)
            nc.vector.memset(ones_sb, 1.0)

            # x tile loads go FIRST (they gate the transpose/projection
            # chain), split across two HWDGE queues (sync + scalar) so the
            # ~0.6us per-DMA issue cost pipelines 2-wide. Weight loads are
            # interleaved on the scalar queue after the first half of x.
            x_tiles = [None] * n_sq
            for t in range(n_sq):
                xt = xin_pool.tile([P, D], FP32, tag="xtile", name=f"xt{t}")
                eng = nc.sync if t % 2 == 0 else nc.scalar
                eng.dma_start(out=xt, in_=x_d[t * P:(t + 1) * P, :])
                x_tiles[t] = xt

            w_sb = {}
            w_dt = {"wq": scores_dtype, "wk": scores_dtype,
                    "wv": scores_dtype, "wo": pv_dtype}
            for name, dram in (("wq", wq_d), ("wk", wk_d), ("wv", wv_d),
                               ("wo", wo_d)):
                for kc in range(n_kc):
                    tf = const_pool.tile([P, D], FP32, name=f"{name}f{kc}")
                    nc.scalar.dma_start(
                        out=tf, in_=dram[kc * P:(kc + 1) * P, :])
                    if w_dt[name] == FP32:
                        w_sb[name, kc] = tf
                    else:
                        t = const_pool.tile([P, D], w_dt[name],
                                            name=f"{name}{kc}")
                        nc.vector.tensor_copy(t, tf)
                        w_sb[name, kc] = t

            # bias broadcast to all 128 partitions (DMA stride-0 read)
            bias_sb = const_pool.tile([P, D], FP32)
            nc.scalar.dma_start(out=bias_sb, in_=bo_d.partition_broadcast(P))

            # ---- persistent big SBUF tensors (mm_dtype) ----
            xT = [big_pool.tile([P, S], scores_dtype, name=f"xT{c}")
                  for c in range(n_kc)]
            qT = [big_pool.tile([P, S], scores_dtype, name=f"qT{c}")
                  for c in range(n_kc)]
            kT = [big_pool.tile([P, S], scores_dtype, name=f"kT{c}")
                  for c in range(n_kc)]
            # vp_all[:, t*260 + h*65 .. +65] = [v_h | 1] for seq tile t.
            # One tile + one early memset: per-tile memsets serialized the
            # v-projection drain chain.
            vp_all = big_pool.tile([P, n_sq * H * (DH + 1)], pv_dtype,
                                   name="vp_all")
            nc.vector.memset(vp_all, 1.0)
            vp = [vp_all[:, t * H * (DH + 1):(t + 1) * H * (DH + 1)]
                  for t in range(n_sq)]
            ctxT = [big_pool.tile([P, S], pv_dtype, name=f"ctxT{c}")
                    for c in range(n_kc)]

            # ---- stage 1: transpose x into xT (cast at drain) ----
            with tc.tile_pool(name="pt", bufs=4, space="PSUM") as pt_pool:
                for t in range(n_sq):
                    xt = x_tiles[t]
                    for c in range(n_kc):
                        ps = pt_pool.tile([P, P], FP32, tag="tr")
                        nc.tensor.transpose(ps, xt[:, c * P:(c + 1) * P], ident)
                        nc.scalar.copy(
                            xT[c][:, t * P:(t + 1) * P], ps)

            # ---- stage 2: projections ----
            with tc.tile_pool(name="pp", bufs=2, space="PSUM") as pp_pool:
                # qT / kT: out[do_chunk][:, :] = sum_kc w[kc][:, do].T @ xT[kc]
                for name, dest in (("wq", qT), ("wk", kT)):
                    for do in range(n_kc):
                        for half in range(2):
                            ps = pp_pool.tile([P, NQ], FP32, tag="proj")
                            for kc in range(n_kc):
                                for nck in range(2):
                                    off = half * NQ + nck * 512
                                    nc.tensor.matmul(
                                        ps[:, nck * 512:(nck + 1) * 512],
                                        lhsT=w_sb[name, kc][:, do * P:(do + 1) * P],
                                        rhs=xT[kc][:, off:off + 512],
                                        start=(kc == 0), stop=(kc == n_kc - 1))
                            nc.vector.tensor_copy(
                                dest[do][:, half * NQ:(half + 1) * NQ], ps)
                # v (natural layout) -> vp tiles
                for t in range(n_sq):
                    ps = pp_pool.tile([P, D], FP32, tag="vproj")
                    for kc in range(n_kc):
                        nc.tensor.matmul(
                            ps, lhsT=xT[kc][:, t * P:(t + 1) * P],
                            rhs=w_sb["wv", kc],
                            start=(kc == 0), stop=(kc == n_kc - 1))
                    for h in range(H):
                        nc.vector.tensor_copy(
                            vp[t][:, h * (DH + 1):h * (DH + 1) + DH],
                            ps[:, h * DH:(h + 1) * DH])

            # ---- stage 3: attention ----
            with (
                tc.tile_pool(name="psc", bufs=2, space="PSUM") as sc_pool,
                tc.tile_pool(name="pctx", bufs=2, space="PSUM") as ctx_pool,
                tc.tile_pool(name="dscr", bufs=2, space="DRAM") as dram_pool,
            ):
                # Software pipeline with lag-1 between scores and PV: the PE
                # FIFO executes in order, so PV(k) (which waits on exp(k) from
                # ACT) must sit BEHIND scores(k+1) in the queue — otherwise
                # every step serializes PE -> ACT -> PE and both engines idle.
                def emit_drain(h, q0, ctx_ps):
                    # normalize rows 0..63 by 1/row64. Reciprocal is a slow
                    # 1-lane DVE op and the broadcast goes via a DRAM
                    # round-trip (stride-0 DRAM reads are legal) — all off
                    # the PE critical path; ctx_pool bufs=2 gives a full
                    # unit of slack before the slot is needed again.
                    recip = drain_pool.tile([1, NQ], FP32, tag="recip")
                    nc.vector.reciprocal(recip, ctx_ps[DH:DH + 1, :])
                    rscr = dram_pool.tile([1, NQ], FP32, tag="rscr")
                    nc.sync.dma_start(out=rscr, in_=recip)
                    recip_b = drain_pool.tile([DH, NQ], FP32, tag="recipb")
                    nc.sync.dma_start(
                        out=recip_b, in_=rscr.partition_broadcast(DH))
                    nc.vector.tensor_mul(
                        ctxT[h // 2][(h % 2) * DH:(h % 2) * DH + DH,
                                     q0:q0 + NQ],
                        ctx_ps[0:DH, :], recip_b)

                steps = [(h, sqh, sk)
                         for h in range(H)
                         for sqh in range(n_sqh)
                         for sk in range(n_sq)]
                ctx_cur = [None]
                pending_pv = [None]

                def emit_pv():
                    if pending_pv[0] is None:
                        return
                    ex, h, sk, q0, ctx_ps = pending_pv[0]
                    pending_pv[0] = None
                    for nck in range(2):
                        nc.tensor.matmul(
                            ctx_ps[:, nck * 512:(nck + 1) * 512],
                            lhsT=vp[sk][:, h * (DH + 1):(h + 1) * (DH + 1)],
                            rhs=ex[:, nck * 512:(nck + 1) * 512],
                            start=(sk == 0), stop=(sk == n_sq - 1))
                    if sk == n_sq - 1:
                        emit_drain(h, q0, ctx_ps)

                for (h, sqh, sk) in steps:
                    qTh = qT[h // 2][(h % 2) * DH:(h % 2) * DH + DH, :]
                    kTh = kT[h // 2][(h % 2) * DH:(h % 2) * DH + DH, :]
                    q0 = sqh * NQ
                    if sk == 0:
                        ctx_cur[0] = ctx_pool.tile(
                            [DH + 1, NQ], FP32, tag="ctx",
                            name=f"ctxps_{h}_{sqh}")
                    ctx_ps = ctx_cur[0]
                    sc = sc_pool.tile([P, NQ], FP32, tag="sc")
                    for nck in range(2):
                        nc.tensor.matmul(
                            sc[:, nck * 512:(nck + 1) * 512],
                            lhsT=kTh[:, sk * P:(sk + 1) * P],
                            rhs=qTh[:, q0 + nck * 512:q0 + (nck + 1) * 512],
                            start=True, stop=True)
                    ex = exp_pool.tile([P, NQ], pv_dtype, tag="exp")
                    nc.scalar.activation(
                        ex, sc, mybir.ActivationFunctionType.Exp,
                        scale=float(SCALE))
                    emit_pv()
                    pending_pv[0] = (ex, h, sk, q0, ctx_ps)
                emit_pv()

            # ---- stage 4: output projection + bias ----
            with tc.tile_pool(name="po", bufs=3, space="PSUM") as po_pool:
                for t in range(n_sq):
                    ps = po_pool.tile([P, D], FP32, tag="oproj")
                    for kc in range(n_kc):
                        nc.tensor.matmul(
                            ps, lhsT=ctxT[kc][:, t * P:(t + 1) * P],
                            rhs=w_sb["wo", kc],
                            start=(kc == 0), stop=(kc == n_kc - 1))
                    ot = out_pool.tile([P, D], FP32, tag="otile")
                    nc.vector.tensor_add(ot, ps, bias_sb)
                    nc.sync.dma_start(
                        out=out_d[t * P:(t + 1) * P, :], in_=ot)

    nc.compile()
    return nc


_NC_CACHE = None


def kernel(x, w_q, w_k, w_v, w_o, b_o):
    global _NC_CACHE
    x = np.ascontiguousarray(np.asarray(x), dtype=np.float32)
    w_q = np.ascontiguousarray(np.asarray(w_q), dtype=np.float32)
    w_k = np.ascontiguousarray(np.asarray(w_k), dtype=np.float32)
    w_v = np.ascontiguousarray(np.asarray(w_v), dtype=np.float32)
    w_o = np.ascontiguousarray(np.asarray(w_o), dtype=np.float32)
    b_o = np.ascontiguousarray(np.asarray(b_o), dtype=np.float32).reshape(1, D)

    if _NC_CACHE is None:
        _NC_CACHE = build_bass()
    nc = _NC_CACHE

    in_maps = []
    for c in range(N_CORES):
        in_maps.append({
            "x": np.ascontiguousarray(x[c]),
            "w_q": w_q, "w_k": w_k, "w_v": w_v, "w_o": w_o, "b_o": b_o,
        })

    res = bass_utils.run_bass_kernel_spmd(
        nc, in_maps, core_ids=list(range(N_CORES)))
    global _LAST_RESULTS
    _LAST_RESULTS = res
    out = np.stack([res.results[c]["out"] for c in range(N_CORES)], axis=0)
    return out


_LAST_RESULTS = None


if __name__ == "__main__":
    rng = np.random.default_rng(0)
    ins = {
        "x": rng.standard_normal((B, S, D), dtype=np.float32),
        "w_q": rng.standard_normal((D, D), dtype=np.float32) / 16,
        "w_k": rng.standard_normal((D, D), dtype=np.float32) / 16,
        "w_v": rng.standard_normal((D, D), dtype=np.float32) / 16,
        "w_o": rng.standard_normal((D, D), dtype=np.float32) / 16,
        "b_o": rng.standard_normal((D,), dtype=np.float32) * 0.01,
    }
    out = kernel(**ins)
    print(out.shape, out.dtype)


# revision 24
# speedup vs baseline: 1.7166x; 1.4696x over previous
"""Multi-head attention (B=8, S=2048, D=256, H=4) on 8 TRN2 NeuronCores.

Sharding: pure data-parallel over batch — core c computes batch element c.
Weights are replicated; no collectives.

Per-core kernel (x: [S, D]):
  xT   = transpose(x)                      (PE transposes, 128x128 tiles)
  qT   = w_q^T @ xT    [D, S]  (transposed layout)
  kT   = w_k^T @ xT    [D, S]
  v    = x @ w_v       [S, D]  (natural layout, stored as vp = [v_h | 1] per head)
  per head h, per sq half:
    scoresT[sk, sq] = kT_h^T(sk-tile) . qT_h   (PE; K=64)
    expT = exp(0.125 * scoresT)                (ACT, one pass, no max-sub:
                                                softmax is shift-invariant and
                                                |s/8| is O(10) => fp32-safe)
    ctxT[65, sq] += vp_h^T(sk-tile) @ expT     (PE; row 64 = softmax denom via
                                                the ones column of vp)
    ctxT_sb = ctxT[0:64] * bcast(1/ctxT[64])   (PE rank-1 bcast + DVE mul;
                                                deferred into the next unit's
                                                sk-loop so PE never stalls on
                                                the 1-lane reciprocal)
  out  = ctx @ w_o + b_o                       (PE over ctxT chunks + DVE add)

Matmul operands are MM_DTYPE (bf16 by default: fp32 matmul on TRN2 runs
2-pass LOW_HIGH with serial un-hidden LDWEIGHTS — ~3x slower). PSUM
accumulation, softmax denominators, reciprocal, broadcast, and the bias
add are always fp32.
"""

import numpy as np

import concourse.bass as bass
import concourse.mybir as mybir
import concourse.tile as tile
from concourse import bacc, bass_utils

B, S, D = 8, 2048, 256
H = 4
DH = D // H          # 64
P = 128              # partitions
SCALE = 1.0 / np.sqrt(DH)  # 0.125
N_CORES = 8

FP32 = mybir.dt.float32
BF16 = mybir.dt.bfloat16
# scores path dtype (x/w_q/w_k/qT/kT) and PV path dtype (vp/expT/ctxT/w_o)
SCORES_DTYPE = BF16
PV_DTYPE = BF16


def build_bass(scores_dtype=None, pv_dtype=None):
    if scores_dtype is None:
        scores_dtype = SCORES_DTYPE
    if pv_dtype is None:
        pv_dtype = PV_DTYPE
    nc = bacc.Bacc("TRN2", target_bir_lowering=False, debug=False,
                   num_devices=N_CORES)

    x_d = nc.dram_tensor("x", [S, D], FP32, kind="ExternalInput").ap()
    wq_d = nc.dram_tensor("w_q", [D, D], FP32, kind="ExternalInput").ap()
    wk_d = nc.dram_tensor("w_k", [D, D], FP32, kind="ExternalInput").ap()
    wv_d = nc.dram_tensor("w_v", [D, D], FP32, kind="ExternalInput").ap()
    wo_d = nc.dram_tensor("w_o", [D, D], FP32, kind="ExternalInput").ap()
    bo_d = nc.dram_tensor("b_o", [1, D], FP32, kind="ExternalInput").ap()
    out_d = nc.dram_tensor("out", [S, D], FP32, kind="ExternalOutput").ap()

    n_sq = S // P        # 16 seq tiles
    n_kc = D // P        # 2 contraction chunks of 128
    NQ = 1024            # sq chunk width for attention inner loop
    n_sqh = S // NQ      # 2

    with tile.TileContext(nc) as tc:
        with (
            tc.tile_pool(name="const", bufs=1) as const_pool,
            tc.tile_pool(name="bigbuf", bufs=1) as big_pool,
            tc.tile_pool(name="xin", bufs=16) as xin_pool,
            tc.tile_pool(name="drain", bufs=3) as drain_pool,
            tc.tile_pool(name="expbuf", bufs=3) as exp_pool,
            tc.tile_pool(name="outbuf", bufs=3) as out_pool,
        ):
            # ---- constants ----
            ident = const_pool.tile([P, P], FP32)
            from concourse.masks import make_identity
            make_identity(nc, ident)
            ones_sb = const_pool.tile([1, DH], FP32)
            nc.vector.memset(ones_sb, 1.0)

            # x tile loads go FIRST (they gate the transpose/projection
            # chain), split across two HWDGE queues (sync + scalar) so the
            # ~0.6us per-DMA issue cost pipelines 2-wide. Weight loads are
            # interleaved on the scalar queue after the first half of x.
            x_tiles = [None] * n_sq
            for t in range(n_sq):
                xt = xin_pool.tile([P, D], FP32, tag="xtile", name=f"xt{t}")
                eng = nc.sync if t % 2 == 0 else nc.scalar
                eng.dma_start(out=xt, in_=x_d[t * P:(t + 1) * P, :])
                x_tiles[t] = xt

            w_sb = {}
            w_dt = {"wq": scores_dtype, "wk": scores_dtype,
                    "wv": scores_dtype, "wo": pv_dtype}
            for name, dram in (("wq", wq_d), ("wk", wk_d), ("wv", wv_d),
                               ("wo", wo_d)):
                for kc in range(n_kc):
                    tf = const_pool.tile([P, D], FP32, name=f"{name}f{kc}")
                    nc.gpsimd.dma_start(
                        out=tf, in_=dram[kc * P:(kc + 1) * P, :])
                    if w_dt[name] == FP32:
                        w_sb[name, kc] = tf
                    else:
                        t = const_pool.tile([P, D], w_dt[name],
                                            name=f"{name}{kc}")
                        nc.vector.tensor_copy(t, tf)
                        w_sb[name, kc] = t

            # bias broadcast to all 128 partitions (DMA stride-0 read)
            bias_sb = const_pool.tile([P, D], FP32)
            nc.gpsimd.dma_start(out=bias_sb, in_=bo_d.partition_broadcast(P))

            # ---- persistent big SBUF tensors (mm_dtype) ----
            xT = [big_pool.tile([P, S], scores_dtype, name=f"xT{c}")
                  for c in range(n_kc)]
            qT = [big_pool.tile([P, S], scores_dtype, name=f"qT{c}")
                  for c in range(n_kc)]
            kT = [big_pool.tile([P, S], scores_dtype, name=f"kT{c}")
                  for c in range(n_kc)]
            # vp_all[:, t*260 + h*65 .. +65] = [v_h | 1] for seq tile t.
            # One tile + one early memset: per-tile memsets serialized the
            # v-projection drain chain.
            vp_all = big_pool.tile([P, n_sq * H * (DH + 1)], pv_dtype,
                                   name="vp_all")
            nc.vector.memset(vp_all, 1.0)
            vp = [vp_all[:, t * H * (DH + 1):(t + 1) * H * (DH + 1)]
                  for t in range(n_sq)]
            ctxT = [big_pool.tile([P, S], pv_dtype, name=f"ctxT{c}")
                    for c in range(n_kc)]

            # ---- stage 1: transpose x into xT (cast at drain) ----
            with tc.tile_pool(name="pt", bufs=6, space="PSUM") as pt_pool:
                for t in range(n_sq):
                    xt = x_tiles[t]
                    for c in range(n_kc):
                        ps = pt_pool.tile([P, P], FP32, tag="tr")
                        nc.tensor.transpose(ps, xt[:, c * P:(c + 1) * P], ident)
                        nc.scalar.copy(
                            xT[c][:, t * P:(t + 1) * P], ps)

            # ---- stage 2: projections ----
            with tc.tile_pool(name="pp", bufs=2, space="PSUM") as pp_pool:
                # qT / kT: out[do_chunk][:, :] = sum_kc w[kc][:, do].T @ xT[kc]
                for name, dest in (("wq", qT), ("wk", kT)):
                    for do in range(n_kc):
                        for half in range(2):
                            ps = pp_pool.tile([P, NQ], FP32, tag="proj")
                            for kc in range(n_kc):
                                for nck in range(2):
                                    off = half * NQ + nck * 512
                                    nc.tensor.matmul(
                                        ps[:, nck * 512:(nck + 1) * 512],
                                        lhsT=w_sb[name, kc][:, do * P:(do + 1) * P],
                                        rhs=xT[kc][:, off:off + 512],
                                        start=(kc == 0), stop=(kc == n_kc - 1))
                            nc.vector.tensor_copy(
                                dest[do][:, half * NQ:(half + 1) * NQ], ps)
                # v (natural layout) -> vp tiles
                for t in range(n_sq):
                    ps = pp_pool.tile([P, D], FP32, tag="vproj")
                    for kc in range(n_kc):
                        nc.tensor.matmul(
                            ps, lhsT=xT[kc][:, t * P:(t + 1) * P],
                            rhs=w_sb["wv", kc],
                            start=(kc == 0), stop=(kc == n_kc - 1))
                    # one strided copy: [128, 4, 64] view of both sides
                    nc.vector.tensor_copy(
                        vp[t].rearrange("p (h j) -> p h j",
                                        j=DH + 1)[:, :, 0:DH],
                        ps.rearrange("p (h j) -> p h j", j=DH))

            # ---- stage 3: attention ----
            with (
                tc.tile_pool(name="psc", bufs=2, space="PSUM") as sc_pool,
                tc.tile_pool(name="pctx", bufs=2, space="PSUM") as ctx_pool,
                tc.tile_pool(name="dscr", bufs=2, space="DRAM") as dram_pool,
            ):
                # Software pipeline with lag-1 between scores and PV: the PE
                # FIFO executes in order, so PV(k) (which waits on exp(k) from
                # ACT) must sit BEHIND scores(k+1) in the queue — otherwise
                # every step serializes PE -> ACT -> PE and both engines idle.
                def emit_drain(h, q0, ctx_ps):
                    # normalize rows 0..63 by 1/row64. Reciprocal is a slow
                    # 1-lane DVE op and the broadcast goes via a DRAM
                    # round-trip (stride-0 DRAM reads are legal) — all off
                    # the PE critical path; ctx_pool bufs=2 gives a full
                    # unit of slack before the slot is needed again.
                    recip = drain_pool.tile([1, NQ], FP32, tag="recip")
                    nc.vector.reciprocal(recip, ctx_ps[DH:DH + 1, :])
                    rscr = dram_pool.tile([1, NQ], FP32, tag="rscr")
                    nc.sync.dma_start(out=rscr, in_=recip)
                    recip_b = drain_pool.tile([DH, NQ], FP32, tag="recipb")
                    nc.sync.dma_start(
                        out=recip_b, in_=rscr.partition_broadcast(DH))
                    nc.vector.tensor_mul(
                        ctxT[h // 2][(h % 2) * DH:(h % 2) * DH + DH,
                                     q0:q0 + NQ],
                        ctx_ps[0:DH, :], recip_b)

                steps = [(h, sqh, sk)
                         for h in range(H)
                         for sqh in range(n_sqh)
                         for sk in range(n_sq)]
                ctx_cur = [None]
                pending_pv = [None]

                def emit_pv():
                    if pending_pv[0] is None:
                        return
                    ex, h, sk, q0, ctx_ps = pending_pv[0]
                    pending_pv[0] = None
                    for nck in range(2):
                        nc.tensor.matmul(
                            ctx_ps[:, nck * 512:(nck + 1) * 512],
                            lhsT=vp[sk][:, h * (DH + 1):(h + 1) * (DH + 1)],
                            rhs=ex[:, nck * 512:(nck + 1) * 512],
                            start=(sk == 0), stop=(sk == n_sq - 1))
                    if sk == n_sq - 1:
                        emit_drain(h, q0, ctx_ps)

                for (h, sqh, sk) in steps:
                    qTh = qT[h // 2][(h % 2) * DH:(h % 2) * DH + DH, :]
                    kTh = kT[h // 2][(h % 2) * DH:(h % 2) * DH + DH, :]
                    q0 = sqh * NQ
                    if sk == 0:
                        ctx_cur[0] = ctx_pool.tile(
                            [DH + 1, NQ], FP32, tag="ctx",
                            name=f"ctxps_{h}_{sqh}")
                    ctx_ps = ctx_cur[0]
                    sc = sc_pool.tile([P, NQ], FP32, tag="sc")
                    for nck in range(2):
                        nc.tensor.matmul(
                            sc[:, nck * 512:(nck + 1) * 512],
                            lhsT=kTh[:, sk * P:(sk + 1) * P],
                            rhs=qTh[:, q0 + nck * 512:q0 + (nck + 1) * 512],
                            start=True, stop=True)
                    ex = exp_pool.tile([P, NQ], pv_dtype, tag="exp")
                    nc.scalar.activation(
                        ex, sc, mybir.ActivationFunctionType.Exp,
                        scale=float(SCALE))
                    emit_pv()
                    pending_pv[0] = (ex, h, sk, q0, ctx_ps)
                emit_pv()

            # ---- stage 4: output projection + bias ----
            with tc.tile_pool(name="po", bufs=3, space="PSUM") as po_pool:
                for t in range(n_sq):
                    ps = po_pool.tile([P, D], FP32, tag="oproj")
                    for kc in range(n_kc):
                        nc.tensor.matmul(
                            ps, lhsT=ctxT[kc][:, t * P:(t + 1) * P],
                            rhs=w_sb["wo", kc],
                            start=(kc == 0), stop=(kc == n_kc - 1))
                    ot = out_pool.tile([P, D], FP32, tag="otile")
                    nc.vector.tensor_add(ot, ps, bias_sb)
                    nc.sync.dma_start(
                        out=out_d[t * P:(t + 1) * P, :], in_=ot)

    nc.compile()
    return nc


_NC_CACHE = None


def kernel(x, w_q, w_k, w_v, w_o, b_o):
    global _NC_CACHE
    x = np.ascontiguousarray(np.asarray(x), dtype=np.float32)
    w_q = np.ascontiguousarray(np.asarray(w_q), dtype=np.float32)
    w_k = np.ascontiguousarray(np.asarray(w_k), dtype=np.float32)
    w_v = np.ascontiguousarray(np.asarray(w_v), dtype=np.float32)
    w_o = np.ascontiguousarray(np.asarray(w_o), dtype=np.float32)
    b_o = np.ascontiguousarray(np.asarray(b_o), dtype=np.float32).reshape(1, D)

    if _NC_CACHE is None:
        _NC_CACHE = build_bass()
    nc = _NC_CACHE

    in_maps = []
    for c in range(N_CORES):
        in_maps.append({
            "x": np.ascontiguousarray(x[c]),
            "w_q": w_q, "w_k": w_k, "w_v": w_v, "w_o": w_o, "b_o": b_o,
        })

    res = bass_utils.run_bass_kernel_spmd(
        nc, in_maps, core_ids=list(range(N_CORES)))
    global _LAST_RESULTS
    _LAST_RESULTS = res
    out = np.stack([res.results[c]["out"] for c in range(N_CORES)], axis=0)
    return out


_LAST_RESULTS = None


if __name__ == "__main__":
    rng = np.random.default_rng(0)
    ins = {
        "x": rng.standard_normal((B, S, D), dtype=np.float32),
        "w_q": rng.standard_normal((D, D), dtype=np.float32) / 16,
        "w_k": rng.standard_normal((D, D), dtype=np.float32) / 16,
        "w_v": rng.standard_normal((D, D), dtype=np.float32) / 16,
        "w_o": rng.standard_normal((D, D), dtype=np.float32) / 16,
        "b_o": rng.standard_normal((D,), dtype=np.float32) * 0.01,
    }
    out = kernel(**ins)
    print(out.shape, out.dtype)
